# revision 1
# baseline (speedup 1.0000x reference)
import math
from functools import partial

import numpy as np
import jax
import jax.numpy as jnp
from jax.sharding import Mesh, PartitionSpec as P

try:
    from jax.experimental.shard_map import shard_map
except ImportError:
    from jax import shard_map

# Problem constants (nn_GQAAttention): B,S,DM = 2,2048,2048; H=32 heads,
# G=8 KV groups, HD=64. TP across the 8 KV groups: each core owns 4 Q
# heads + 1 KV group; W_QKV rows and W_O cols split contiguously by group.
B, S, DM = 2, 2048, 2048
H, G, HD = 32, 8, 64
HPG = H // G
Q_DIM = H * HD      # 2048
KV_DIM = G * HD     # 512
SCALE = 1.0 / math.sqrt(HD)


def _shard_fn(x, wq, wk, wv, wo, mask):
    # x [B,S,DM] replicated; wq [Q_DIM/G, DM]; wk,wv [HD, DM]; wo [DM, Q_DIM/G]
    q = (x @ wq.T).reshape(B, S, HPG, HD).transpose(0, 2, 1, 3)  # [B,HPG,S,HD]
    k = x @ wk.T                                                  # [B,S,HD]
    v = x @ wv.T
    scores = jnp.einsum("bhqd,bkd->bhqk", q, k) * SCALE
    scores = jnp.where(mask == 0, jnp.float32(-1e9), scores)
    probs = jax.nn.softmax(scores, axis=-1)
    o = jnp.einsum("bhqk,bkd->bhqd", probs, v)
    o = o.transpose(0, 2, 1, 3).reshape(B, S, HPG * HD)
    part = o @ wo.T                                               # [B,S,DM]
    return jax.lax.psum(part, "tp")


_JITTED = None


def _get_fn():
    global _JITTED
    if _JITTED is None:
        mesh = Mesh(np.array(jax.devices()[:8]), ("tp",))
        fn = shard_map(
            _shard_fn,
            mesh=mesh,
            in_specs=(
                P(None, None, None),
                P("tp", None),
                P("tp", None),
                P("tp", None),
                P(None, "tp"),
                P(None, None, None, None),
            ),
            out_specs=P(None, None, None),
        )
        _JITTED = jax.jit(fn)
    return _JITTED


def kernel(input_, W_QKV, W_O, attention_mask):
    fn = _get_fn()
    wq = jnp.asarray(W_QKV[:Q_DIM])
    wk = jnp.asarray(W_QKV[Q_DIM : Q_DIM + KV_DIM])
    wv = jnp.asarray(W_QKV[Q_DIM + KV_DIM :])
    out = fn(
        jnp.asarray(input_),
        wq,
        wk,
        wv,
        jnp.asarray(W_O),
        jnp.asarray(attention_mask),
    )
    return np.asarray(jax.device_get(out), dtype=np.float32)


# revision 2
# speedup vs baseline: 1.1592x; 1.1592x over previous
import math
from functools import partial

import numpy as np
import jax
import jax.numpy as jnp
from jax.sharding import Mesh, PartitionSpec as P

try:
    from jax.experimental.shard_map import shard_map
except ImportError:
    from jax import shard_map

# Problem constants (nn_GQAAttention): B,S,DM = 2,2048,2048; H=32 heads,
# G=8 KV groups, HD=64. TP across the 8 KV groups: each core owns 4 Q
# heads + 1 KV group; W_QKV rows and W_O cols split contiguously by group.
B, S, DM = 2, 2048, 2048
H, G, HD = 32, 8, 64
HPG = H // G
Q_DIM = H * HD      # 2048
KV_DIM = G * HD     # 512
SCALE = 1.0 / math.sqrt(HD)


def _shard_fn(x, wq, wk, wv, wo, mask):
    # x [B,S,DM] replicated; wq [Q_DIM/G, DM]; wk,wv [HD, DM]; wo [DM, Q_DIM/G]
    q = (x @ wq.T).reshape(B, S, HPG, HD).transpose(0, 2, 1, 3)  # [B,HPG,S,HD]
    k = x @ wk.T                                                  # [B,S,HD]
    v = x @ wv.T
    scores = jnp.einsum("bhqd,bkd->bhqk", q, k) * SCALE
    scores = jnp.where(mask == 0, jnp.float32(-1e9), scores)
    probs = jax.nn.softmax(scores, axis=-1)
    o = jnp.einsum("bhqk,bkd->bhqd", probs, v)
    o = o.transpose(0, 2, 1, 3).reshape(B, S, HPG * HD)
    part = o @ wo.T                                               # [B,S,DM]
    return jax.lax.psum(part, "tp")


_JITTED = None


def _get_fn():
    global _JITTED
    if _JITTED is None:
        mesh = Mesh(np.array(jax.devices()[:8]), ("tp",))
        fn = shard_map(
            _shard_fn,
            mesh=mesh,
            in_specs=(
                P(None, None, None),
                P("tp", None),
                P("tp", None),
                P("tp", None),
                P(None, "tp"),
                P(None, None, None, None),
            ),
            out_specs=P(None, None, None),
        )
        _JITTED = jax.jit(fn)
    return _JITTED


def kernel(input_, W_QKV, W_O, attention_mask):
    fn = _get_fn()
    wq = jnp.asarray(W_QKV[:Q_DIM])
    wk = jnp.asarray(W_QKV[Q_DIM : Q_DIM + KV_DIM])
    wv = jnp.asarray(W_QKV[Q_DIM + KV_DIM :])
    out = fn(
        jnp.asarray(input_),
        wq,
        wk,
        wv,
        jnp.asarray(W_O),
        jnp.asarray(np.asarray(attention_mask).astype(np.int8)),
    )
    return np.asarray(jax.device_get(out), dtype=np.float32)
